# revision 1
# baseline (speedup 1.0000x reference)
"""Trainium2 Bass kernel for nn_CapsuleLayer (B=32, In=128, Din=256, ch=32, Nc=47, Dc=64).

Sharding: over the OUTPUT-CAPSULE axis Nc (47 -> pad 48 = 8 cores x 6 capsules).
Routing is fully independent per (batch, output-capsule), and W (94 MiB) is the
dominant HBM tensor -- Nc-sharding reads W exactly once total (12.6 MiB/core)
instead of replicating it 8x as batch-sharding would.

Per-core layout ("layout R"): inputs_hat stored [p=(b,rr) 128 partitions,
(c, n, k) 12288 free] -- produced directly by per-channel matmuls
ih_c[(b,rr),(n,k)] = sum_d xT_c[d,(b,rr)]^T . WT_c[d,(n,k)].
Partition-group (rr / b-group) reductions run on the PE with block-diagonal
ones matrices (BD4 [128,32], BD4T [32,128]); free-dim (c, k) reductions on the
DVE.  Routing iteration t:
  a  = sum_k OUT_{t-1} * ih        (DVE mul + reduce-X over k)
  E  = exp(sum_t a)                (ACT; softmax normalization folded into s)
  s  = (sum_i E*ih)/Z + B          (DVE mul + reduce-X over c, PE BD4 matmul)
  OUT = squash(s)                  (small [32,384] ops)
Iteration 1 (uniform c) is s1 = BD4^T IH summed over channels on the PE.

Toolchain constraint: matmul (S3_LW) and DMA (DIRECT2D) instructions accept at
most ONE sync wait at codegen.  Hence: the x|w stream is 8 one-shot DMAs into
disjoint regions of one resident SBUF tile (no slot WAR/WAW, one queue each);
both matmul operands come from the same stream region (one DMA sem); const
DMAs ride SWDGE and are pre-absorbed into the PE clock via tiny dummy matmuls;
psum-slot WAR ticks are absorbed the same way.  The routing scratch TMP
aliases the stream tile (dead after phase 1).
"""

import numpy as np

B, IN, DIN = 32, 128, 256
CH, NC, DC = 32, 47, 64
NCP = 48          # padded Nc
NSH = 6           # capsules per core
NCORES = 8
NK = NSH * DC     # 384
EPS = 1e-7

_cache = {}


def _build_nc():
    import concourse.bass as bass
    import concourse.tile as tile
    from concourse import mybir
    from concourse.tile_rust import add_dep_helper

    f32 = mybir.dt.float32
    nc = bass.Bass()

    # packed stream: per cd-chunk rows [xt(128) | wt(384)] = 512 floats
    xw = nc.dram_tensor("xw", [CH * 2, 128, 512], f32, kind="ExternalInput")
    # consts packed in one tensor: [bd4(0:32) | bd4t(rows0:32, 32:160) |
    # brep(rows0:32, 160:544)]
    cst = nc.dram_tensor("cst", [128, 544], f32, kind="ExternalInput")
    out_d = nc.dram_tensor("out", [B, NK], f32, kind="ExternalOutput")

    ADD = mybir.AluOpType.add
    MULT = mybir.AluOpType.mult
    AX = mybir.AxisListType.X
    AF = mybir.ActivationFunctionType

    with tile.TileContext(nc) as tc:
        with (
            tc.tile_pool(name="singles", bufs=1) as singles,
            tc.tile_pool(name="work", bufs=1) as work,
            tc.tile_pool(name="small", bufs=2) as small,
            tc.tile_pool(name="ps_ih", bufs=2, space="PSUM") as ps_ih,
            tc.tile_pool(name="ps_s1", bufs=1, space="PSUM") as ps_s1,
            tc.tile_pool(name="ps_rep", bufs=1, space="PSUM") as ps_rep,
            tc.tile_pool(name="ps_sm", bufs=1, space="PSUM") as ps_sm,
            tc.tile_pool(name="ps_s2", bufs=2, space="PSUM") as ps_s2,
        ):
            cst_t = singles.tile([128, 544], f32)
            c_dma = nc.sync.dma_start(out=cst_t[:], in_=cst[:])
            bd4_t = cst_t[:, 0:B]
            bd4t_t = cst_t[0:B, B:B + 128]
            brep_t = cst_t[0:B, B + 128:B + 128 + NK]
            eps_t = singles.tile([B, 1], f32)
            nc.vector.memset(eps_t[:], EPS)
            # DVE is also 1-wait-limited: pre-observe the const-DMA sem
            dve_scratch = singles.tile([B, 8], f32)
            nc.vector.tensor_copy(dve_scratch[:, 0:2], cst_t[:B, :2])

            IH = singles.tile([128, CH, NK], f32)
            # resident packed stream; 8 one-shot DMAs into disjoint regions
            STREAM = singles.tile([128, CH * 2, 512], f32)
            # routing scratch aliases the stream area (dead after phase 1)
            TMP = (STREAM[:]
                   .rearrange("p a b -> p (a b)")[:, :CH * NK]
                   .rearrange("p (c nk) -> p c nk", c=CH))

            # Absorb the const-DMA sem into the PE clock (dummy matmul) so
            # real matmuls carry a single wait.
            pd = ps_sm.tile([2, 2], f32, tag="dummy")
            last_dummy = nc.tensor.matmul(
                pd[:], cst_t[:2, :2], cst_t[:2, :2], start=True, stop=True,
                skip_group_check=True,
            )

            psum_s1 = ps_s1.tile([B, NK], f32)

            # ---------------- phase 1: inputs_hat + iter-1 s ----------------
            # 2 big stream DMAs: each splits across all 16 SDMA engine slots
            # internally (full HBM BW), while keeping the kernel-tail drain's
            # semaphore count low (wait-slot limit on the drain).
            s_dmas = []
            for g in range(2):
                s_dmas.append(nc.sync.dma_start(
                    out=STREAM[:, 32 * g:32 * (g + 1), :],
                    in_=xw[32 * g:32 * (g + 1)].rearrange("q d f -> d q f"),
                ))
            # DVE pre-observes the stream-DMA sems (TMP aliases the stream)
            stream_scratch = singles.tile([128, 16], f32)
            for g in range(2):
                nc.vector.tensor_copy(stream_scratch[:, 2 * g:2 * g + 2],
                                      STREAM[:, 32 * g, :2])
            copy_insts = []
            for c in range(CH):
                if c >= 2:
                    # absorb the psum-slot WAR tick (copy of c-2, DVE/ACT sem)
                    dmy = nc.tensor.matmul(pd[:], bd4_t[:, :2], bd4_t[:, :2],
                                           start=True, stop=True,
                                           skip_group_check=True)
                    add_dep_helper(dmy.ins, copy_insts[c - 2].ins, sync=True,
                                   reason="absorb psum WAR tick on PE")
                    last_dummy = dmy
                psum_ih = ps_ih.tile([128, NK], f32, tag="ih")
                for dc in range(2):
                    cd = c * 2 + dc
                    mih = nc.tensor.matmul(
                        psum_ih[:], STREAM[:, cd, 0:128], STREAM[:, cd, 128:512],
                        start=(dc == 0), stop=(dc == 1),
                    )
                    if dc == 0:
                        add_dep_helper(mih.ins, last_dummy.ins, sync=False,
                                       reason="order dummy before matmul")
                # spread PSUM->SBUF copies across DVE and ACT
                if c % 2 == 0:
                    copy_insts.append(nc.vector.tensor_copy(IH[:, c, :], psum_ih[:]))
                else:
                    copy_insts.append(nc.scalar.copy(IH[:, c, :], psum_ih[:]))
                # iter-1 s accumulation: sum_rr over partitions via BD4, and
                # over channels via PSUM accumulation
                nc.tensor.matmul(
                    psum_s1[:], bd4_t[:], IH[:, c, :],
                    start=(c == 0), stop=(c == CH - 1),
                    skip_group_check=True,
                )

            _absn = [0]

            def absorb(eng, src_ap, target_hint=""):
                """Tiny copy on `eng` reading src_ap: pre-observes the
                producer's sem so the next real op keeps a single wait."""
                _absn[0] += 1
                scr = small.tile([2, 2], f32, tag="abs%d" % _absn[0])
                if eng == "v":
                    return nc.vector.tensor_copy(scr[:], src_ap)
                return nc.scalar.copy(scr[:], src_ap)

            def squash(S, tag):
                """S: [B, NK] sbuf tile -> OUT [B, NK] sbuf tile."""
                Ssq = work.tile([B, NK], f32, tag="Su")
                nc.vector.tensor_mul(Ssq[:], S[:], S[:])
                m2 = small.tile([B, NSH], f32, tag="m2")
                nc.vector.tensor_reduce(
                    m2[:],
                    Ssq[:].rearrange("p (n k) -> p n k", n=NSH),
                    axis=AX, op=ADD,
                )
                d1 = small.tile([B, NSH], f32, tag="d1")
                nc.vector.tensor_scalar_add(d1[:], m2[:], 1.0)
                absorb("s", m2[:2, :2])          # ACT clock <- m2 (DVE)
                sq = small.tile([B, NSH], f32, tag="sq")
                nc.scalar.activation(sq[:], m2[:], AF.Sqrt, bias=eps_t[:])
                absorb("v", sq[:2, :2])          # DVE clock <- sq (ACT)
                den = small.tile([B, NSH], f32, tag="den")
                nc.vector.tensor_mul(den[:], d1[:], sq[:])
                rden = small.tile([B, NSH], f32, tag="rden")
                nc.vector.reciprocal(rden[:], den[:])
                g_ = small.tile([B, NSH], f32, tag="g")
                nc.vector.tensor_mul(g_[:], m2[:], rden[:])
                OUT = work.tile([B, NK], f32, tag="out")
                nc.vector.tensor_mul(
                    OUT[:].rearrange("p (n k) -> p n k", n=NSH),
                    S[:].rearrange("p (n k) -> p n k", n=NSH),
                    g_[:].rearrange("p (n o) -> p n o", o=1).broadcast_to([B, NSH, DC]),
                )
                return OUT

            def replicate(OUT, tag):
                """OUT [B, NK] -> [128, NK] (row b to partitions 4b..4b+3)."""
                pr = ps_rep.tile([128, NK], f32, tag="rep")
                nc.tensor.matmul(pr[:], bd4t_t[:], OUT[:], start=True, stop=True)
                R = work.tile([128, NK], f32, tag="rep")
                nc.vector.tensor_copy(R[:], pr[:])
                return R

            # ---------------- iter 1 ----------------
            S1 = work.tile([B, NK], f32, tag="S")
            nc.vector.scalar_tensor_tensor(
                out=S1[:], in0=psum_s1[:], scalar=1.0 / IN, in1=brep_t[:],
                op0=MULT, op1=ADD,
            )
            OUT1 = squash(S1, "1")
            OUTr = replicate(OUT1, "1")

            Aprev = None
            for it in (2, 3):
                # a-step: A = sum_k OUTr * IH   -> [128, CH, NSH]
                absorb("v", IH[:2, CH - 1, :2])   # DVE clock <- last ACT copy
                nc.vector.tensor_mul(
                    TMP,
                    IH[:],
                    OUTr[:].rearrange("p (o nk) -> p o nk", o=1)
                          .broadcast_to([128, CH, NK]),
                )
                A = work.tile([128, CH, NSH], f32, tag="A%d" % it)
                nc.vector.tensor_reduce(
                    A[:],
                    TMP.rearrange("p c (n k) -> p c n k", n=NSH),
                    axis=AX, op=ADD,
                )
                if Aprev is None:
                    BL = A
                    Aprev = A
                else:
                    BL = work.tile([128, CH, NSH], f32, tag="BL")
                    nc.vector.tensor_add(BL[:], A[:], Aprev[:])
                # E = exp(BL)
                absorb("s", BL[:2, 0, :2])        # ACT clock <- BL (DVE)
                E = work.tile([128, CH, NSH], f32, tag="E")
                nc.scalar.activation(E[:], BL[:], AF.Exp)
                # Zp[p, n] = sum_c E ; Z = BD4 reduction over rr
                absorb("v", E[:2, 0, :2])         # DVE clock <- E (ACT)
                Zp = small.tile([128, NSH], f32, tag="Zp")
                nc.vector.tensor_reduce(
                    Zp[:],
                    E[:].rearrange("p c n -> p n c"),
                    axis=AX, op=ADD,
                )
                pz = ps_sm.tile([B, NSH], f32, tag="pz")
                nc.tensor.matmul(pz[:], bd4_t[:], Zp[:], start=True, stop=True)
                # s-step: TMP = E*IH ; P2 = sum_c ; S = BD4(P2)/Z + brep
                nc.vector.tensor_mul(
                    TMP.rearrange("p c (n k) -> p c n k", n=NSH),
                    IH[:].rearrange("p c (n k) -> p c n k", n=NSH),
                    E[:].rearrange("p c (n o) -> p c n o", o=1)
                       .broadcast_to([128, CH, NSH, DC]),
                )
                P2 = work.tile([128, NK], f32, tag="P2")
                nc.vector.tensor_reduce(
                    P2[:].rearrange("p (n k) -> p n k", n=NSH),
                    TMP.rearrange("p c (n k) -> p n k c", n=NSH),
                    axis=AX, op=ADD,
                )
                pS = ps_s2.tile([B, NK], f32, tag="pS")
                mm_last = nc.tensor.matmul(pS[:], bd4_t[:], P2[:],
                                           start=True, stop=True)
                Zs = small.tile([B, NSH], f32, tag="Zs")
                nc.vector.tensor_copy(Zs[:], pz[:])
                Rz = small.tile([B, NSH], f32, tag="Rz")
                nc.vector.reciprocal(Rz[:], Zs[:])
                absorb("v", pS[:2, :2])          # DVE clock <- pS (PE)
                Su = work.tile([B, NK], f32, tag="Su")
                nc.vector.tensor_mul(
                    Su[:].rearrange("p (n k) -> p n k", n=NSH),
                    pS[:].rearrange("p (n k) -> p n k", n=NSH),
                    Rz[:].rearrange("p (n o) -> p n o", o=1).broadcast_to([B, NSH, DC]),
                )
                S = work.tile([B, NK], f32, tag="S")
                nc.vector.tensor_add(S[:], Su[:], brep_t[:])
                OUT = squash(S, str(it))
                if it < 3:
                    OUTr = replicate(OUT, str(it))
                else:
                    o_dma = nc.sync.dma_start(out=out_d[:], in_=OUT[:])
                    # Pre-absorb every final sem into the SYNC engine so the
                    # Tile kernel-tail drain needs <=1 wait (codegen limit).
                    f_scr = small.tile([2, 4], f32, tag="fin")
                    f_act = nc.scalar.copy(f_scr[:, 0:2], OUT[:2, :2])
                    f_dve = nc.vector.tensor_copy(f_scr[:, 2:4], OUT[:2, :2])
                    for fin in (c_dma, s_dmas[0], s_dmas[1], mm_last,
                                f_act, f_dve, o_dma):
                        fnop = nc.sync.nop()
                        add_dep_helper(fnop.ins, fin.ins, sync=True,
                                       reason="absorb final sem for tail drain")

    return nc


def _pack_inputs(inputs, W, B_param):
    """Host-side shard + relayout. Returns list of 8 in_maps."""
    inputs = np.ascontiguousarray(inputs, dtype=np.float32)
    W = np.ascontiguousarray(W, dtype=np.float32)
    B_param = np.ascontiguousarray(B_param, dtype=np.float32)

    Wp = np.zeros((CH, NCP, DC, DIN), dtype=np.float32)
    Wp[:, :NC] = W
    Bp = np.zeros((NCP, DC), dtype=np.float32)
    Bp[:NC] = B_param

    # xt[(c,dc), dd, (b,rr)] = x[b, 4c+rr, 128dc+dd]
    x4 = inputs.reshape(B, CH, 4, 2, 128)           # b, c, rr, dc, dd
    xt = x4.transpose(1, 3, 4, 0, 2).reshape(CH * 2, 128, 128)
    bd4 = np.zeros((128, B), dtype=np.float32)
    bd4[np.arange(128), np.arange(128) // 4] = 1.0
    bd4t = bd4.T

    in_maps = []
    for core in range(NCORES):
        sl = slice(core * NSH, (core + 1) * NSH)
        Wc = Wp[:, sl]                               # c, n, k, d
        w5 = Wc.reshape(CH, NSH, DC, 2, 128)         # c n k dc dd
        wtc = w5.transpose(0, 3, 4, 1, 2).reshape(CH * 2, 128, NK)
        cstc = np.zeros((128, 544), dtype=np.float32)
        cstc[:, 0:B] = bd4
        cstc[0:B, B:B + 128] = bd4t
        cstc[0:B, B + 128:B + 128 + NK] = np.broadcast_to(
            Bp[sl].reshape(1, NK), (B, NK))
        xwc = np.concatenate([xt, wtc], axis=2)      # [64, 128, 512]
        in_maps.append(dict(xw=np.ascontiguousarray(xwc), cst=cstc))
    return in_maps


def _run(inputs, W, B_param, trace=False):
    from concourse.bass_utils import run_bass_kernel_spmd

    if "nc" not in _cache:
        _cache["nc"] = _build_nc()
    nc = _cache["nc"]
    in_maps = _pack_inputs(inputs, W, B_param)
    res = run_bass_kernel_spmd(nc, in_maps, core_ids=list(range(NCORES)),
                               trace=trace)
    outs = [r["out"].reshape(B, NSH, DC) for r in res.results]
    full = np.concatenate(outs, axis=1)[:, :NC, :]
    return np.ascontiguousarray(full.astype(np.float32)), res


def kernel(inputs, W, B_param):
    out, _ = _run(inputs, W, B_param, trace=False)
    return out



# revision 6
# speedup vs baseline: 1.5949x; 1.5949x over previous
"""Trainium2 Bass kernel for nn_CapsuleLayer (B=32, In=128, Din=256, ch=32, Nc=47, Dc=64).

Sharding: over the OUTPUT-CAPSULE axis Nc (47 -> pad 48 = 8 cores x 6 capsules).
W (94 MiB) is the dominant HBM tensor -- Nc-sharding reads W exactly once total.

bf16 pipeline (rel_err ~6e-3 vs 2e-2 gate):
- stream (x|W) in bf16, partition-major HBM layout -> 32KB-contiguous DMA runs
- inputs_hat via bf16 matmuls (1 cy/row vs fp32's 4)
- IH stored TWICE from PSUM: k-inner [p,(c,n,k)] for the a-step and c-inner
  [p,(n,k,c)] for the s-step, so both big DVE muls hit the 2x bf16 perf mode
  (packed innermost operands; measured 0.64 ns/col vs 1.28 broadcast/1x)
- reductions as pairwise bf16 tree-adds (2x) instead of TENSOR_REDUCE (1x)

Routing iteration t (per core, Nsh=6 capsules):
  TMP  = IH * OUTr            (DVE 2x, k-inner)
  A    = tree-fold k 64->1    (DVE 2x, last level fp32)
  E    = exp(sum_t A)         (ACT, written transposed to [p,(n,c)])
  Zp   = reduce_c E           (DVE, into SCRATCH[384:390])
  TMP2 = IHC * E              (DVE 2x, c-inner)
  P2   = tree-fold c 32->1    (DVE 2x, into SCRATCH[0:384])
  pS   = BD4^T [P2|Zp]        (PE partition reduce over (b,rr))
  S    = pS/Z + Brep ; OUT = squash(S)  (small [32,384] ops)
Iteration 1 (uniform c): S1 = psum_s1/IN + Brep via PSUM-accumulated
BD4^T IH_c matmuls during phase 1.

Toolchain constraint: EVERY engine instruction accepts at most ONE sync wait
at codegen.  Same-engine deps are free (program order / one monotonic sem per
engine); cross-engine fan-in is handled by absorb ops (tiny reads that
pre-observe a sem) and dummy matmuls on the PE.
"""

import numpy as np

B, IN, DIN = 32, 128, 256
CH, NC, DC = 32, 47, 64
NCP = 48          # padded Nc
NSH = 6           # capsules per core
NCORES = 8
NK = NSH * DC     # 384
EPS = 1e-7

_cache = {}


def _build_nc():
    import concourse.bass as bass
    import concourse.tile as tile
    from concourse import mybir
    from concourse.tile_rust import add_dep_helper

    f32 = mybir.dt.float32
    bf = mybir.dt.bfloat16
    nc = bass.Bass()

    # partition-major packed stream: xw[d, cd, 0:128]=xT, [128:512]=wT (bf16)
    xw = nc.dram_tensor("xw", [128, CH * 2, 512], bf, kind="ExternalInput")
    # consts: [bd4(0:32) | bd4t(rows0:32, 32:160) | brep(rows0:32, 160:544)]
    cst = nc.dram_tensor("cst", [128, 544], bf, kind="ExternalInput")
    out_d = nc.dram_tensor("out", [B, NK], f32, kind="ExternalOutput")

    ADD = mybir.AluOpType.add
    MULT = mybir.AluOpType.mult
    AX = mybir.AxisListType.X
    AF = mybir.ActivationFunctionType

    with tile.TileContext(nc) as tc:
        with (
            tc.tile_pool(name="singles", bufs=1) as singles,
            tc.tile_pool(name="work", bufs=1) as work,
            tc.tile_pool(name="small", bufs=2) as small,
            tc.tile_pool(name="ps_ih", bufs=2, space="PSUM") as ps_ih,
            tc.tile_pool(name="ps_s1", bufs=1, space="PSUM") as ps_s1,
            tc.tile_pool(name="ps_s", bufs=2, space="PSUM") as ps_s,
            tc.tile_pool(name="ps_rep", bufs=2, space="PSUM") as ps_rep,
            tc.tile_pool(name="ps_d", bufs=1, space="PSUM") as ps_d,
        ):
            cst_t = singles.tile([128, 544], bf)
            c_dma = nc.sync.dma_start(out=cst_t[:], in_=cst[:])
            bd4_t = cst_t[:, 0:B]                 # [128, 32] bf16
            bd4t_t = cst_t[0:B, B:B + 128]        # [32, 128] bf16
            brep_t = cst_t[0:B, B + 128:B + 128 + NK]   # [32, 384] bf16
            eps_t = singles.tile([B, 1], f32)
            nc.vector.memset(eps_t[:], EPS)
            # DVE/ACT pre-observe the const-DMA sem
            dve_scratch = singles.tile([4, 8], bf)
            nc.vector.tensor_copy(dve_scratch[:2, 0:2], cst_t[:2, :2])
            act_scratch = singles.tile([4, 8], bf)
            nc.scalar.copy(act_scratch[:2, 0:2], cst_t[:2, :2])

            IH = singles.tile([128, CH, NK], bf)      # k-inner
            IHC = singles.tile([128, NK, CH], bf)     # c-inner
            STREAM = singles.tile([128, CH * 2, 512], bf)
            TMP = singles.tile([128, CH * NK], bf)    # mul product scratch
            U1 = singles.tile([128, 6144], bf)
            U2 = singles.tile([128, 3072], bf)
            SCR = singles.tile([128, NK + NSH], bf)   # [P2 | Zp]
            A2 = singles.tile([128, CH * NSH], f32)
            A3 = singles.tile([128, CH * NSH], f32)
            E = singles.tile([128, NSH * CH], bf)     # [p, (n, c)]
            OUTr = singles.tile([128, NK], bf)

            # Absorb the const-DMA sem into the PE clock (dummy matmul).
            pd = ps_d.tile([2, 2], f32, tag="dummy")
            last_dummy = nc.tensor.matmul(
                pd[:], cst_t[:2, :2], cst_t[:2, :2], start=True, stop=True,
                skip_group_check=True,
            )

            psum_s1 = ps_s1.tile([B, NK], f32)

            # ---------------- phase 1: inputs_hat + iter-1 s ----------------
            s_dmas = []
            for g in range(2):
                s_dmas.append(nc.sync.dma_start(
                    out=STREAM[:, 32 * g:32 * (g + 1), :],
                    in_=xw[:, 32 * g:32 * (g + 1), :],
                ))
            copy_last = []      # last psum reader per c (engine-alternating)
            for c in range(CH):
                if c >= 2:
                    # absorb the psum-slot WAR tick into the PE clock
                    dmy = nc.tensor.matmul(pd[:], bd4_t[:2, :2], bd4_t[:2, :2],
                                           start=True, stop=True,
                                           skip_group_check=True)
                    add_dep_helper(dmy.ins, copy_last[c - 2].ins, sync=True,
                                   reason="absorb psum WAR tick on PE")
                    last_dummy = dmy
                psum_ih = ps_ih.tile([128, NK], f32, tag="ih")
                for dc in range(2):
                    cd = c * 2 + dc
                    mih = nc.tensor.matmul(
                        psum_ih[:], STREAM[:, cd, 0:128], STREAM[:, cd, 128:512],
                        start=(dc == 0), stop=(dc == 1),
                    )
                    if dc == 0:
                        add_dep_helper(mih.ins, last_dummy.ins, sync=False,
                                       reason="order dummy before matmul")
                # both copies for channel c on ONE engine (alternating) so the
                # WAR absorb needs a single sem
                if c % 2 == 0:
                    nc.vector.tensor_copy(IH[:, c, :], psum_ih[:])
                    cl = nc.vector.tensor_copy(IHC[:, :, c], psum_ih[:])
                else:
                    nc.scalar.copy(IH[:, c, :], psum_ih[:])
                    cl = nc.scalar.copy(IHC[:, :, c], psum_ih[:])
                copy_last.append(cl)
                # iter-1 s accumulation (PSUM accumulate over c)
                nc.tensor.matmul(
                    psum_s1[:], bd4_t[:], IH[:, c, :],
                    start=(c == 0), stop=(c == CH - 1),
                    skip_group_check=True,
                )

            _absn = [0]

            def absorb(eng, src_ap):
                """Tiny copy on `eng` reading src_ap: pre-observes the
                producer's sem so the next real op keeps a single wait."""
                _absn[0] += 1
                scr = small.tile([2, 2], f32, tag="abs%d" % _absn[0])
                if eng == "v":
                    return nc.vector.tensor_copy(scr[:], src_ap)
                return nc.scalar.copy(scr[:], src_ap)

            def squash(S, it):
                """S: [B, NK] f32 sbuf tile -> OUT tile (bf16 it<3, f32 it=3)."""
                Ssq = work.tile([B, NK], f32, tag="Su")
                nc.vector.tensor_mul(Ssq[:], S[:], S[:])
                m2 = small.tile([B, NSH], f32, tag="m2")
                nc.vector.tensor_reduce(
                    m2[:], Ssq[:].rearrange("p (n k) -> p n k", n=NSH),
                    axis=AX, op=ADD,
                )
                d1 = small.tile([B, NSH], f32, tag="d1")
                nc.vector.tensor_scalar_add(d1[:], m2[:], 1.0)
                absorb("s", m2[:2, :2])          # ACT clock <- m2 (DVE)
                sq = small.tile([B, NSH], f32, tag="sq")
                nc.scalar.activation(sq[:], m2[:], AF.Sqrt, bias=eps_t[:])
                absorb("v", sq[:2, :2])          # DVE clock <- sq (ACT)
                den = small.tile([B, NSH], f32, tag="den")
                nc.vector.tensor_mul(den[:], d1[:], sq[:])
                rden = small.tile([B, NSH], f32, tag="rden")
                nc.vector.reciprocal(rden[:], den[:])
                g_ = small.tile([B, NSH], f32, tag="g")
                nc.vector.tensor_mul(g_[:], m2[:], rden[:])
                OUT = work.tile([B, NK], f32 if it == 3 else bf,
                                tag="out%d" % it)
                nc.vector.tensor_mul(
                    OUT[:].rearrange("p (n k) -> p n k", n=NSH),
                    S[:].rearrange("p (n k) -> p n k", n=NSH),
                    g_[:].rearrange("p (n o) -> p n o", o=1)
                        .broadcast_to([B, NSH, DC]),
                )
                return OUT

            def replicate(OUTb, it):
                """OUTb [B, NK] bf16 -> OUTr [128, NK] bf16 (row b -> 4b..4b+3)."""
                pr = ps_rep.tile([128, NK], f32, tag="rep")
                mm = nc.tensor.matmul(pr[:], bd4t_t[:], OUTb[:],
                                      start=True, stop=True)
                cp = nc.vector.tensor_copy(OUTr[:], pr[:])
                return mm, cp

            # ---------------- iter 1 ----------------
            S1 = work.tile([B, NK], f32, tag="S")
            nc.vector.scalar_tensor_tensor(
                out=S1[:], in0=psum_s1[:], scalar=1.0 / IN, in1=brep_t[:],
                op0=MULT, op1=ADD,
            )
            OUT1 = squash(S1, 1)
            rep_mm, rep_cp = replicate(OUT1, 1)

            TMPk = TMP[:].rearrange("p (c n k) -> p c n k", c=CH, n=NSH)
            TMPc = TMP[:].rearrange("p (n k c) -> p n k c", n=NSH, k=DC)
            U1k = U1[:].rearrange("p (c n k) -> p c n k", c=CH, n=NSH)
            U2k = U2[:].rearrange("p (c n k) -> p c n k", c=CH, n=NSH)
            U1c = U1[:].rearrange("p (n k c) -> p n k c", n=NSH, k=DC)
            U2c = U2[:].rearrange("p (n k c) -> p n k c", n=NSH, k=DC)

            for it in (2, 3):
                # ---- a-step: TMP = IH * OUTr ; A = tree-fold k ----
                nc.vector.tensor_mul(
                    TMP[:].rearrange("p (c nk) -> p c nk", c=CH),
                    IH[:].rearrange("p c nk -> p c nk"),
                    OUTr[:].rearrange("p (o nk) -> p o nk", o=1)
                          .broadcast_to([128, CH, NK]),
                )
                nc.vector.tensor_add(U1k[:, :, :, 0:32], TMPk[:, :, :, 0:32],
                                     TMPk[:, :, :, 32:64])
                nc.vector.tensor_add(U2k[:, :, :, 0:16], U1k[:, :, :, 0:16],
                                     U1k[:, :, :, 16:32])
                nc.vector.tensor_add(U1k[:, :, :, 0:8], U2k[:, :, :, 0:8],
                                     U2k[:, :, :, 8:16])
                nc.vector.tensor_add(U2k[:, :, :, 0:4], U1k[:, :, :, 0:4],
                                     U1k[:, :, :, 4:8])
                nc.vector.tensor_add(U1k[:, :, :, 0:2], U2k[:, :, :, 0:2],
                                     U2k[:, :, :, 2:4])
                At = A2 if it == 2 else A3
                nc.vector.tensor_add(
                    At[:].rearrange("p (c n o) -> p c n o", c=CH, o=1),
                    U1k[:, :, :, 0:1], U1k[:, :, :, 1:2],
                )
                if it == 2:
                    BL = A2
                else:
                    BL = A3
                    nc.vector.tensor_add(A3[:], A3[:], A2[:])
                # ---- E = exp(BL), transposed write to [p, (n, c)] ----
                absorb("s", At[:2, :2])         # ACT clock <- tree (DVE)
                nc.scalar.activation(
                    E[:].rearrange("p (n c) -> p c n", n=NSH),
                    BL[:].rearrange("p (c n) -> p c n", c=CH),
                    AF.Exp,
                )
                # ---- Zp = sum_c E -> SCR[384:390] ----
                absorb("v", E[:2, :2])          # DVE clock <- E (ACT)
                with nc.allow_low_precision(reason="Z normalizer, positive sum"):
                    nc.vector.tensor_reduce(
                        SCR[:, NK:NK + NSH],
                        E[:].rearrange("p (n c) -> p n c", n=NSH),
                        axis=AX, op=ADD,
                    )
                # ---- s-step: TMP2 = IHC * E ; P2 = tree-fold c ----
                nc.vector.tensor_mul(
                    TMPc,
                    IHC[:].rearrange("p nk c -> p nk c")
                       .rearrange("p (n k) c -> p n k c", n=NSH),
                    E[:].rearrange("p (n o c) -> p n o c", n=NSH, o=1)
                       .broadcast_to([128, NSH, DC, CH]),
                )
                nc.vector.tensor_add(U1c[:, :, :, 0:16], TMPc[:, :, :, 0:16],
                                     TMPc[:, :, :, 16:32])
                nc.vector.tensor_add(U2c[:, :, :, 0:8], U1c[:, :, :, 0:8],
                                     U1c[:, :, :, 8:16])
                nc.vector.tensor_add(U1c[:, :, :, 0:4], U2c[:, :, :, 0:4],
                                     U2c[:, :, :, 4:8])
                nc.vector.tensor_add(U2c[:, :, :, 0:2], U1c[:, :, :, 0:2],
                                     U1c[:, :, :, 2:4])
                nc.vector.tensor_add(
                    SCR[:, 0:NK].rearrange("p (n k o) -> p n k o", n=NSH, o=1),
                    U2c[:, :, :, 0:1], U2c[:, :, :, 1:2],
                )
                # ---- pS = BD4^T [P2|Zp] ----
                pS = ps_s.tile([B, NK + NSH], f32, tag="pS")
                mm_last = nc.tensor.matmul(pS[:], bd4_t[:], SCR[:],
                                           start=True, stop=True)
                # ---- S = pS/Z + brep ----
                absorb("v", pS[:2, :2])         # DVE clock <- pS (PE)
                Rz = small.tile([B, NSH], f32, tag="Rz")
                nc.vector.reciprocal(Rz[:], pS[:, NK:NK + NSH])
                Su = work.tile([B, NK], f32, tag="Su2")
                nc.vector.tensor_mul(
                    Su[:].rearrange("p (n k) -> p n k", n=NSH),
                    pS[:, 0:NK].rearrange("p (n k) -> p n k", n=NSH),
                    Rz[:].rearrange("p (n o) -> p n o", o=1)
                        .broadcast_to([B, NSH, DC]),
                )
                S = work.tile([B, NK], f32, tag="S")
                nc.vector.tensor_add(S[:], Su[:], brep_t[:])
                OUT = squash(S, it)
                if it < 3:
                    rep_mm, rep_cp = replicate(OUT, it)
                else:
                    o_dma = nc.sync.dma_start(out=out_d[:], in_=OUT[:])
                    f_scr = small.tile([2, 4], f32, tag="fin")
                    f_act = nc.scalar.copy(f_scr[:, 0:2], OUT[:2, :2])
                    f_dve = nc.vector.tensor_copy(f_scr[:, 2:4], OUT[:2, :2])
                    for fin in (c_dma, s_dmas[0], s_dmas[1], mm_last,
                                f_act, f_dve, o_dma):
                        fnop = nc.sync.nop()
                        add_dep_helper(fnop.ins, fin.ins, sync=True,
                                       reason="absorb final sem for tail drain")

    return nc


def _pack_inputs(inputs, W, B_param):
    """Host-side shard + relayout. Returns list of 8 in_maps."""
    import ml_dtypes
    bf16 = ml_dtypes.bfloat16
    inputs = np.ascontiguousarray(inputs, dtype=np.float32)
    W = np.ascontiguousarray(W, dtype=np.float32)
    B_param = np.ascontiguousarray(B_param, dtype=np.float32)

    Wp = np.zeros((CH, NCP, DC, DIN), dtype=np.float32)
    Wp[:, :NC] = W
    Bp = np.zeros((NCP, DC), dtype=np.float32)
    Bp[:NC] = B_param

    # xt[(c,dc), dd, (b,rr)] = x[b, 4c+rr, 128dc+dd]
    x4 = inputs.reshape(B, CH, 4, 2, 128)           # b, c, rr, dc, dd
    xt = x4.transpose(1, 3, 4, 0, 2).reshape(CH * 2, 128, 128)
    bd4 = np.zeros((128, B), dtype=np.float32)
    bd4[np.arange(128), np.arange(128) // 4] = 1.0
    bd4t = bd4.T

    in_maps = []
    for core in range(NCORES):
        sl = slice(core * NSH, (core + 1) * NSH)
        Wc = Wp[:, sl]                               # c, n, k, d
        w5 = Wc.reshape(CH, NSH, DC, 2, 128)         # c n k dc dd
        wtc = w5.transpose(0, 3, 4, 1, 2).reshape(CH * 2, 128, NK)
        cstc = np.zeros((128, 544), dtype=np.float32)
        cstc[:, 0:B] = bd4
        cstc[0:B, B:B + 128] = bd4t
        cstc[0:B, B + 128:B + 128 + NK] = np.broadcast_to(
            Bp[sl].reshape(1, NK), (B, NK))
        xwc = np.concatenate([xt, wtc], axis=2)      # [64, 128, 512]
        # partition-major: [d, cd, 512] for 32KB-contiguous DMA runs
        xwc = np.ascontiguousarray(xwc.transpose(1, 0, 2)).astype(bf16)
        in_maps.append(dict(xw=xwc, cst=cstc.astype(bf16)))
    return in_maps


def _run(inputs, W, B_param, trace=False):
    from concourse.bass_utils import run_bass_kernel_spmd

    if "nc" not in _cache:
        _cache["nc"] = _build_nc()
    nc = _cache["nc"]
    in_maps = _pack_inputs(inputs, W, B_param)
    res = run_bass_kernel_spmd(nc, in_maps, core_ids=list(range(NCORES)),
                               trace=trace)
    outs = [r["out"].reshape(B, NSH, DC) for r in res.results]
    full = np.concatenate(outs, axis=1)[:, :NC, :]
    return np.ascontiguousarray(full.astype(np.float32)), res


def kernel(inputs, W, B_param):
    out, _ = _run(inputs, W, B_param, trace=False)
    return out


# revision 12
# speedup vs baseline: 1.8850x; 1.1818x over previous
"""Trainium2 Bass kernel for nn_CapsuleLayer (B=32, In=128, Din=256, ch=32, Nc=47, Dc=64).

Sharding: over the OUTPUT-CAPSULE axis Nc (47 -> pad 48 = 8 cores x 6 capsules).
W (94 MiB) is the dominant HBM tensor -- Nc-sharding reads W exactly once total.

bf16 pipeline (rel_err ~6e-3 vs 2e-2 gate):
- stream (x|W) in bf16, partition-major HBM layout -> 32KB-contiguous DMA runs
- inputs_hat via bf16 matmuls (1 cy/row vs fp32's 4)
- IH stored TWICE from PSUM: k-inner [p,(c,n,k)] for the a-step and c-inner
  [p,(n,k,c)] for the s-step, so both big DVE muls hit the 2x bf16 perf mode
  (packed innermost operands; measured 0.64 ns/col vs 1.28 broadcast/1x)
- reductions as pairwise bf16 tree-adds (2x) instead of TENSOR_REDUCE (1x)

Routing iteration t (per core, Nsh=6 capsules):
  TMP  = IH * OUTr            (DVE 2x, k-inner)
  A    = tree-fold k 64->1    (DVE 2x, last level fp32)
  E    = exp(sum_t A)         (ACT, written transposed to [p,(n,c)])
  Zp   = reduce_c E           (DVE, into SCRATCH[384:390])
  TMP2 = IHC * E              (DVE 2x, c-inner)
  P2   = tree-fold c 32->1    (DVE 2x, into SCRATCH[0:384])
  pS   = BD4^T [P2|Zp]        (PE partition reduce over (b,rr))
  S    = pS/Z + Brep ; OUT = squash(S)  (small [32,384] ops)
Iteration 1 (uniform c): S1 = psum_s1/IN + Brep via PSUM-accumulated
BD4^T IH_c matmuls during phase 1.

Toolchain constraint: EVERY engine instruction accepts at most ONE sync wait
at codegen.  Same-engine deps are free (program order / one monotonic sem per
engine); cross-engine fan-in is handled by absorb ops (tiny reads that
pre-observe a sem) and dummy matmuls on the PE.
"""

import numpy as np

B, IN, DIN = 32, 128, 256
CH, NC, DC = 32, 47, 64
NCP = 48          # padded Nc
NSH = 6           # capsules per core
NCORES = 8
NK = NSH * DC     # 384
EPS = 1e-7

_cache = {}


def _build_nc():
    import concourse.bass as bass
    import concourse.tile as tile
    from concourse import mybir
    from concourse.tile_rust import add_dep_helper

    f32 = mybir.dt.float32
    bf = mybir.dt.bfloat16
    nc = bass.Bass()

    # partition-major packed stream: xw[d, cd, 0:128]=xT, [128:512]=wT (bf16)
    xw = nc.dram_tensor("xw", [128, CH * 2, 512], bf, kind="ExternalInput")
    # consts: [bd4(0:32) | bd4t(rows0:32, 32:160) | brep(rows0:32, 160:544)]
    cst = nc.dram_tensor("cst", [128, 544], bf, kind="ExternalInput")
    out_d = nc.dram_tensor("out", [B, NK], f32, kind="ExternalOutput")

    ADD = mybir.AluOpType.add
    MULT = mybir.AluOpType.mult
    AX = mybir.AxisListType.X
    AF = mybir.ActivationFunctionType

    with tile.TileContext(nc) as tc:
        with (
            tc.tile_pool(name="singles", bufs=1) as singles,
            tc.tile_pool(name="work", bufs=1) as work,
            tc.tile_pool(name="small", bufs=2) as small,
            tc.tile_pool(name="ps_ih", bufs=3, space="PSUM") as ps_ih,
            tc.tile_pool(name="ps_s1", bufs=1, space="PSUM") as ps_s1,
            tc.tile_pool(name="ps_s", bufs=2, space="PSUM") as ps_s,
            tc.tile_pool(name="ps_rep", bufs=2, space="PSUM") as ps_rep,
        ):
            cst_t = singles.tile([128, 544], bf)
            c_dma = nc.sync.dma_start(out=cst_t[:], in_=cst[:])
            bd4_t = cst_t[:, 0:B]                 # [128, 32] bf16
            bd4t_t = cst_t[0:B, B:B + 128]        # [32, 128] bf16
            brep_t = cst_t[0:B, B + 128:B + 128 + NK]   # [32, 384] bf16
            eps_t = singles.tile([B, 1], f32)
            nc.vector.memset(eps_t[:], EPS)
            # DVE/ACT pre-observe the const-DMA sem
            dve_scratch = singles.tile([4, 8], bf)
            nc.vector.tensor_copy(dve_scratch[:2, 0:2], cst_t[:2, :2])
            act_scratch = singles.tile([4, 8], bf)
            nc.scalar.copy(act_scratch[:2, 0:2], cst_t[:2, :2])
            act_f32 = singles.tile([4, 2], f32)
            nc.scalar.activation(act_f32[:2, 0:2], act_scratch[:2, 0:2],
                                 AF.Exp)

            IH = singles.tile([128, CH, NK], bf)      # k-inner
            IHC = singles.tile([128, NK, CH + 1], bf)  # c-inner, pad stride 33
            STREAM = singles.tile([128, CH * 2, 512], bf)
            TMP = singles.tile([128, CH * NK], bf)    # mul product scratch
            U1 = singles.tile([128, 6144], bf)
            U2 = singles.tile([128, 3072], bf)
            SCR = singles.tile([128, NK + NSH], bf)   # [P2 | Zp]
            A2 = singles.tile([128, CH * NSH], f32)
            A3 = singles.tile([128, CH * NSH], f32)
            E = singles.tile([128, NSH * CH], bf)     # [p, (n, c)]
            OUTr = singles.tile([128, NK], bf)

            # Absorb the const-DMA sem into the PE clock (PE nop).
            last_dummy = nc.tensor.nop()
            add_dep_helper(last_dummy.ins, c_dma.ins, sync=True,
                           reason="absorb cst DMA sem into PE clock")

            psum_s1 = ps_s1.tile([B, NK], f32)

            # ---------------- phase 1: inputs_hat + iter-1 s ----------------
            s_dmas = []
            for g in range(4):
                s_dmas.append(nc.sync.dma_start(
                    out=STREAM[:, 16 * g:16 * (g + 1), :],
                    in_=xw[:, 16 * g:16 * (g + 1), :],
                ))
            copy_last = []      # last psum reader per c (engine-alternating)
            ih_done = []

            def s1_mm(c):
                nc.tensor.matmul(
                    psum_s1[:], bd4_t[:], IH[:, c, :],
                    start=(c == 0), stop=(c == CH - 1),
                    skip_group_check=True,
                )

            for c in range(CH):
                if c >= 3:
                    # absorb the psum-slot WAR tick into the PE clock
                    dmy = nc.tensor.nop()
                    add_dep_helper(dmy.ins, copy_last[c - 3].ins, sync=True,
                                   reason="absorb psum WAR tick on PE")
                    last_dummy = dmy
                psum_ih = ps_ih.tile([128, NK], f32, tag="ih")
                for dc in range(2):
                    cd = c * 2 + dc
                    mih = nc.tensor.matmul(
                        psum_ih[:], STREAM[:, cd, 0:128], STREAM[:, cd, 128:512],
                        start=(dc == 0), stop=(dc == 1),
                    )
                    if dc == 0:
                        add_dep_helper(mih.ins, last_dummy.ins, sync=False,
                                       reason="order dummy before matmul")
                # both copies for channel c on ONE engine (alternating) so the
                # WAR absorb needs a single sem
                if c % 2 == 0:
                    nc.vector.tensor_copy(IH[:, c, :], psum_ih[:])
                    cl = nc.vector.tensor_copy(IHC[:, :, c], psum_ih[:])
                else:
                    nc.scalar.copy(IH[:, c, :], psum_ih[:])
                    cl = nc.scalar.copy(IHC[:, :, c], psum_ih[:])
                copy_last.append(cl)
                # iter-1 s accumulation, lagged 2 channels behind the copies
                if c >= 2:
                    s1_mm(c - 2)
            s1_mm(CH - 2)
            s1_mm(CH - 1)

            _absn = [0]

            def absorb(eng, src_ap):
                """Tiny copy on `eng` reading src_ap: pre-observes the
                producer's sem so the next real op keeps a single wait."""
                _absn[0] += 1
                scr = small.tile([2, 2], f32, tag="abs%d" % _absn[0])
                if eng == "v":
                    return nc.vector.tensor_copy(scr[:], src_ap)
                return nc.scalar.copy(scr[:], src_ap)

            def squash(S, it):
                """S: [B, NK] f32 sbuf tile -> OUT tile (bf16 it<3, f32 it=3)."""
                Ssq = work.tile([B, NK], f32, tag="Su")
                nc.vector.tensor_mul(Ssq[:], S[:], S[:])
                m2 = small.tile([B, NSH], f32, tag="m2")
                nc.vector.tensor_reduce(
                    m2[:], Ssq[:].rearrange("p (n k) -> p n k", n=NSH),
                    axis=AX, op=ADD,
                )
                d1 = small.tile([B, NSH], f32, tag="d1")
                nc.vector.tensor_scalar_add(d1[:], m2[:], 1.0)
                rd1 = small.tile([B, NSH], f32, tag="rd1")
                nc.vector.reciprocal(rd1[:], d1[:])
                absorb("s", m2[:2, :2])          # ACT clock <- m2 (DVE)
                # rsqrt(m2+eps) = exp(-0.5*ln(m2+eps)); ln+exp share one
                # ACT table set (no SQRT table thrash)
                ln_ = small.tile([B, NSH], f32, tag="ln")
                nc.scalar.activation(ln_[:], m2[:], AF.Ln, bias=eps_t[:])
                rsq = small.tile([B, NSH], f32, tag="rsq")
                nc.scalar.activation(rsq[:], ln_[:], AF.Exp, scale=-0.5)
                absorb("v", rsq[:2, :2])         # DVE clock <- rsq (ACT)
                t_ = small.tile([B, NSH], f32, tag="t")
                nc.vector.tensor_mul(t_[:], m2[:], rsq[:])
                g_ = small.tile([B, NSH], f32, tag="g")
                nc.vector.tensor_mul(g_[:], t_[:], rd1[:])
                OUT = work.tile([B, NK], f32 if it == 3 else bf,
                                tag="out%d" % it)
                nc.vector.tensor_mul(
                    OUT[:].rearrange("p (n k) -> p n k", n=NSH),
                    S[:].rearrange("p (n k) -> p n k", n=NSH),
                    g_[:].rearrange("p (n o) -> p n o", o=1)
                        .broadcast_to([B, NSH, DC]),
                )
                return OUT

            def replicate(OUTb, it):
                """OUTb [B, NK] bf16 -> OUTr [128, NK] bf16 (row b -> 4b..4b+3)."""
                pr = ps_rep.tile([128, NK], f32, tag="rep")
                mm = nc.tensor.matmul(pr[:], bd4t_t[:], OUTb[:],
                                      start=True, stop=True)
                cp = nc.vector.tensor_copy(OUTr[:], pr[:])
                return mm, cp

            # ---------------- iter 1 ----------------
            S1 = work.tile([B, NK], f32, tag="S")
            nc.vector.scalar_tensor_tensor(
                out=S1[:], in0=psum_s1[:], scalar=1.0 / IN, in1=brep_t[:],
                op0=MULT, op1=ADD,
            )
            OUT1 = squash(S1, 1)
            rep_mm, rep_cp = replicate(OUT1, 1)

            TMPk = TMP[:].rearrange("p (c n k) -> p c n k", c=CH, n=NSH)
            TMPc = TMP[:].rearrange("p (n k c) -> p n k c", n=NSH, k=DC)
            U1k = U1[:].rearrange("p (c n k) -> p c n k", c=CH, n=NSH)
            U2k = U2[:].rearrange("p (c n k) -> p c n k", c=CH, n=NSH)
            U1c = U1[:].rearrange("p (n k c) -> p n k c", n=NSH, k=DC)
            U2c = U2[:].rearrange("p (n k c) -> p n k c", n=NSH, k=DC)

            for it in (2, 3):
                # ---- a-step: TMP = IH * OUTr ; A = tree-fold k ----
                nc.vector.tensor_mul(
                    TMP[:].rearrange("p (c nk) -> p c nk", c=CH),
                    IH[:].rearrange("p c nk -> p c nk"),
                    OUTr[:].rearrange("p (o nk) -> p o nk", o=1)
                          .broadcast_to([128, CH, NK]),
                )
                nc.vector.tensor_add(U1k[:, :, :, 0:32], TMPk[:, :, :, 0:32],
                                     TMPk[:, :, :, 32:64])
                nc.vector.tensor_add(U2k[:, :, :, 0:16], U1k[:, :, :, 0:16],
                                     U1k[:, :, :, 16:32])
                nc.vector.tensor_add(U1k[:, :, :, 0:8], U2k[:, :, :, 0:8],
                                     U2k[:, :, :, 8:16])
                nc.vector.tensor_add(U2k[:, :, :, 0:4], U1k[:, :, :, 0:4],
                                     U1k[:, :, :, 4:8])
                nc.vector.tensor_add(U1k[:, :, :, 0:2], U2k[:, :, :, 0:2],
                                     U2k[:, :, :, 2:4])
                At = A2 if it == 2 else A3
                nc.vector.tensor_add(
                    At[:].rearrange("p (c n o) -> p c n o", c=CH, o=1),
                    U1k[:, :, :, 0:1], U1k[:, :, :, 1:2],
                )
                if it == 2:
                    BL = A2
                else:
                    BL = A3
                    nc.vector.tensor_add(A3[:], A3[:], A2[:])
                # ---- E = exp(BL), transposed write to [p, (n, c)] ----
                absorb("s", At[:2, :2])         # ACT clock <- tree (DVE)
                nc.scalar.activation(
                    E[:].rearrange("p (n c) -> p c n", n=NSH),
                    BL[:].rearrange("p (c n) -> p c n", c=CH),
                    AF.Exp,
                )
                # ---- Zp = sum_c E -> SCR[384:390] ----
                absorb("v", E[:2, :2])          # DVE clock <- E (ACT)
                with nc.allow_low_precision(reason="Z normalizer, positive sum"):
                    nc.vector.tensor_reduce(
                        SCR[:, NK:NK + NSH],
                        E[:].rearrange("p (n c) -> p n c", n=NSH),
                        axis=AX, op=ADD,
                    )
                # ---- s-step: TMP2 = IHC * E ; P2 = tree-fold c ----
                nc.vector.tensor_mul(
                    TMPc,
                    IHC[:, :, 0:CH]
                       .rearrange("p (n k) c -> p n k c", n=NSH),
                    E[:].rearrange("p (n o c) -> p n o c", n=NSH, o=1)
                       .broadcast_to([128, NSH, DC, CH]),
                )
                nc.vector.tensor_add(U1c[:, :, :, 0:16], TMPc[:, :, :, 0:16],
                                     TMPc[:, :, :, 16:32])
                nc.vector.tensor_add(U2c[:, :, :, 0:8], U1c[:, :, :, 0:8],
                                     U1c[:, :, :, 8:16])
                nc.vector.tensor_add(U1c[:, :, :, 0:4], U2c[:, :, :, 0:4],
                                     U2c[:, :, :, 4:8])
                nc.vector.tensor_add(U2c[:, :, :, 0:2], U1c[:, :, :, 0:2],
                                     U1c[:, :, :, 2:4])
                nc.vector.tensor_add(
                    SCR[:, 0:NK].rearrange("p (n k o) -> p n k o", n=NSH, o=1),
                    U2c[:, :, :, 0:1], U2c[:, :, :, 1:2],
                )
                # ---- pS = BD4^T [P2|Zp] ----
                pS = ps_s.tile([B, NK + NSH], f32, tag="pS")
                mm_last = nc.tensor.matmul(pS[:], bd4_t[:], SCR[:],
                                           start=True, stop=True)
                # ---- S = pS/Z + brep ----
                absorb("v", pS[:2, :2])         # DVE clock <- pS (PE)
                Rz = small.tile([B, NSH], f32, tag="Rz")
                nc.vector.reciprocal(Rz[:], pS[:, NK:NK + NSH])
                Su = work.tile([B, NK], f32, tag="Su2")
                nc.vector.tensor_mul(
                    Su[:].rearrange("p (n k) -> p n k", n=NSH),
                    pS[:, 0:NK].rearrange("p (n k) -> p n k", n=NSH),
                    Rz[:].rearrange("p (n o) -> p n o", o=1)
                        .broadcast_to([B, NSH, DC]),
                )
                S = work.tile([B, NK], f32, tag="S")
                nc.vector.tensor_add(S[:], Su[:], brep_t[:])
                OUT = squash(S, it)
                if it < 3:
                    rep_mm, rep_cp = replicate(OUT, it)
                else:
                    # absorb stream/cst DMA queue sems into SYNC first so the
                    # out-DMA's queue-reuse wait dedups to a single sem
                    for fin in (c_dma, *s_dmas):
                        fnop = nc.sync.nop()
                        add_dep_helper(fnop.ins, fin.ins, sync=True,
                                       reason="absorb DMA sem for queue reuse")
                    o_dma = nc.sync.dma_start(out=out_d[:], in_=OUT[:])
                    f_scr = small.tile([2, 4], f32, tag="fin")
                    f_act = nc.scalar.copy(f_scr[:, 0:2], OUT[:2, :2])
                    f_dve = nc.vector.tensor_copy(f_scr[:, 2:4], OUT[:2, :2])
                    for fin in (mm_last, f_act, f_dve, o_dma):
                        fnop = nc.sync.nop()
                        add_dep_helper(fnop.ins, fin.ins, sync=True,
                                       reason="absorb final sem for tail drain")

    return nc


def _pack_inputs(inputs, W, B_param):
    """Host-side shard + relayout. Returns list of 8 in_maps."""
    import ml_dtypes
    bf16 = ml_dtypes.bfloat16
    inputs = np.ascontiguousarray(inputs, dtype=np.float32)
    W = np.ascontiguousarray(W, dtype=np.float32)
    B_param = np.ascontiguousarray(B_param, dtype=np.float32)

    Wp = np.zeros((CH, NCP, DC, DIN), dtype=np.float32)
    Wp[:, :NC] = W
    Bp = np.zeros((NCP, DC), dtype=np.float32)
    Bp[:NC] = B_param

    # xt[(c,dc), dd, (b,rr)] = x[b, 4c+rr, 128dc+dd]
    x4 = inputs.reshape(B, CH, 4, 2, 128)           # b, c, rr, dc, dd
    xt = x4.transpose(1, 3, 4, 0, 2).reshape(CH * 2, 128, 128)
    bd4 = np.zeros((128, B), dtype=np.float32)
    bd4[np.arange(128), np.arange(128) // 4] = 1.0
    bd4t = bd4.T

    in_maps = []
    for core in range(NCORES):
        sl = slice(core * NSH, (core + 1) * NSH)
        Wc = Wp[:, sl]                               # c, n, k, d
        w5 = Wc.reshape(CH, NSH, DC, 2, 128)         # c n k dc dd
        wtc = w5.transpose(0, 3, 4, 1, 2).reshape(CH * 2, 128, NK)
        cstc = np.zeros((128, 544), dtype=np.float32)
        cstc[:, 0:B] = bd4
        cstc[0:B, B:B + 128] = bd4t
        cstc[0:B, B + 128:B + 128 + NK] = np.broadcast_to(
            Bp[sl].reshape(1, NK), (B, NK))
        xwc = np.concatenate([xt, wtc], axis=2)      # [64, 128, 512]
        # partition-major: [d, cd, 512] for 32KB-contiguous DMA runs
        xwc = np.ascontiguousarray(xwc.transpose(1, 0, 2)).astype(bf16)
        in_maps.append(dict(xw=xwc, cst=cstc.astype(bf16)))
    return in_maps


def _run(inputs, W, B_param, trace=False):
    from concourse.bass_utils import run_bass_kernel_spmd

    if "nc" not in _cache:
        _cache["nc"] = _build_nc()
    nc = _cache["nc"]
    in_maps = _pack_inputs(inputs, W, B_param)
    res = run_bass_kernel_spmd(nc, in_maps, core_ids=list(range(NCORES)),
                               trace=trace)
    outs = [r["out"].reshape(B, NSH, DC) for r in res.results]
    full = np.concatenate(outs, axis=1)[:, :NC, :]
    return np.ascontiguousarray(full.astype(np.float32)), res


def kernel(inputs, W, B_param):
    out, _ = _run(inputs, W, B_param, trace=False)
    return out
